# revision 45
# baseline (speedup 1.0000x reference)
"""MultiHeadLatentAttention Trainium2 kernel (8 NeuronCores, SPMD).

Sharding: batch (2) x head-group (4 of 4 heads each) -> 8 cores.
Each core computes, for its batch b and heads [4g, 4g+4):
  phase 1 (merged, bf16): latT = Wkv^T x^T + bkv AND QT = Wq_sl^T x^T + bq
    in one pass over x^T tiles (8 PSUM banks: 4 lat + 4 q).
  phase 2 (bf16): KT = Wk_sl^T latT + bk ; V = latT^T Wv_sl (no bias --
    A rows sum to 1 so bv folds into the host-side bo via bo + bv @ Wo).
  phase 3 (f32r): per head h, q-chunk j (512 wide), k-tile i2:
    causal trim: diagonal k-tile r=i2-4j only covers q in [128r, 512).
    ST[k,q] = KT_h[:,i2]^T QT_h[:,j]  on the live region
    diag block masked via one [128,128] triangular NEG add (DVE)
    ET = exp(scale*ST) (ACT); ET tiles presummed on DVE into acc;
    OT[dh,q] += V_h[i2]^T ET ; then ONE colsum matmul ones^T acc,
    reciprocal + rank-1 broadcast, OTs = OT * (1/colsum).
  phase 4 (f32r): y_partial[q,:] = sum_h OTs_h[:,qtile]^T Wo_sl_h;
    PSUM->SBUF copy on ACT, DMA out on sync.
Host: y[b] = sum of 4 partials + (bo + bv @ Wo).

Matmul dtype rule (empirical): the MOVING operand of an f32r matmul must be
produced by instructions writing f32r-typed APs; the stationary operand can
be f32-written and bitcast at the matmul. bf16 ops have no such rule.
"""
import sys

sys.path.insert(0, "/opt/trn_rl_repo")

import math
import numpy as np
import ml_dtypes

import concourse.bass as bass
import concourse.mybir as mybir
import concourse.tile as tile
from concourse import bacc
from concourse.bass_utils import run_bass_kernel_spmd

F32 = mybir.dt.float32
F32R = mybir.dt.float32r
BF16 = mybir.dt.bfloat16
FP16 = mybir.dt.float16
EXP = mybir.ActivationFunctionType.Exp

D_MODEL = 2048
NUM_HEADS = 16
D_HEAD = 128
D_LATENT = 512
B, S = 2, 2048
N_CORES = 8
HG = 4                      # head-groups (cores per batch)
HPC = NUM_HEADS // HG       # heads per core = 4
HSL = HPC * D_HEAD          # head-dim slice per core = 512
NQT = S // 128              # 16 q tiles of 128
NJ = S // 512               # 4 q chunks of 512
NKK = D_MODEL // 128        # 16 contraction chunks
NLK = D_LATENT // 128       # 4 latent chunks
SCALE = 1.0 / math.sqrt(D_HEAD)
NEG = -1.0e9

_BUILD_CACHE = {}


def build(causal: bool):
    if causal in _BUILD_CACHE:
        return _BUILD_CACHE[causal]
    nc = bacc.Bacc("TRN2", target_bir_lowering=False, debug=False,
                   num_devices=N_CORES)

    # weights arrive host-pretransposed to [128 partitions, ...] so each
    # lands in one contiguous large-line DMA
    xT = nc.dram_tensor("xT", [D_MODEL, S], BF16, kind="ExternalInput").ap()
    wq = nc.dram_tensor("wq", [D_MODEL, HSL], BF16, kind="ExternalInput").ap()
    bq = nc.dram_tensor("bq", [HPC, 128], F32, kind="ExternalInput").ap()
    wkv = nc.dram_tensor("wkv", [D_MODEL, D_LATENT], BF16,
                         kind="ExternalInput").ap()
    bkv = nc.dram_tensor("bkv", [NLK, 128], F32, kind="ExternalInput").ap()
    wk = nc.dram_tensor("wk", [128, NLK * HSL], BF16, kind="ExternalInput").ap()
    bk = nc.dram_tensor("bk", [HPC, 128], F32, kind="ExternalInput").ap()
    wv = nc.dram_tensor("wv", [128, NLK * HSL], BF16, kind="ExternalInput").ap()
    wo = nc.dram_tensor("wo", [128, HPC * D_MODEL], F32, kind="ExternalInput").ap()
    if causal:
        maskb = nc.dram_tensor("maskb", [128, 128], F32,
                               kind="ExternalInput").ap()
    else:
        maskb = nc.dram_tensor("maskb", [S, S], F32, kind="ExternalInput").ap()
    y = nc.dram_tensor("y", [S, D_MODEL], BF16, kind="ExternalOutput").ap()

    xTr = xT.rearrange("(kk p) s -> kk p s", p=128)

    with tile.TileContext(nc) as tc:
        from contextlib import ExitStack
        with ExitStack() as ctx:
            # ---- pools, strictly LIFO lifetimes ----
            consts = ctx.enter_context(tc.tile_pool(name="consts", bufs=1))
            persist = ctx.enter_context(tc.tile_pool(name="persist", bufs=1))
            qt_sb = persist.tile([128, HPC, S], F32R, tag="qt")
            kv_pool = ctx.enter_context(tc.tile_pool(name="kvp", bufs=1))
            kt_sb = kv_pool.tile([128, HPC, S], F32R, tag="kt")
            v_sb = kv_pool.tile([128, NQT, HSL], FP16, tag="v")
            wk_sb = kv_pool.tile([128, NLK, HSL], BF16, tag="wk")
            wv_sb = kv_pool.tile([128, NLK, HSL], BF16, tag="wv")
            wo_sb = kv_pool.tile([128, HPC, D_MODEL], F32R, tag="wo")

            ones1 = consts.tile([1, 128], F32, tag="ones1")
            onesk = consts.tile([128, 1], F32, tag="onesk")
            ones1h = consts.tile([1, 128], FP16, tag="ones1h")
            oneskh = consts.tile([128, 1], FP16, tag="oneskh")
            bq_sb = consts.tile([128, HPC], F32, tag="bq")
            bkv_sb = consts.tile([128, NLK], F32, tag="bkv")
            bk_sb = consts.tile([128, HPC], F32, tag="bk")
            if causal:
                mb_sb = consts.tile([128, 128], F32, tag="mb")

            warm_mv = consts.tile([1, 512], FP16, tag="warm")
            nc.vector.memset(ones1, 1.0)
            nc.vector.memset(onesk, 1.0)
            nc.vector.memset(ones1h, 1.0)
            nc.vector.memset(oneskh, 1.0)
            nc.vector.memset(warm_mv, 1.0)

            lat_sb = kv_pool.tile([128, NLK, S], BF16, tag="lat")

            # ---- phase 1 (merged): latT = Wkv^T xT + bkv ; QT = Wq^T xT + bq
            with tc.tile_pool(name="p1w", bufs=1) as p1w, \
                 tc.tile_pool(name="p1x", bufs=10) as p1x, \
                 tc.tile_pool(name="p1p", bufs=8, space="PSUM") as p1p:
                wkv_sb = p1w.tile([128, NKK, D_LATENT], BF16, tag="wkv")
                wq_sb = p1w.tile([128, NKK, HSL], BF16, tag="wq")
                wkvr = wkv.rearrange("(kk p) m -> kk p m", p=128)
                wqr = wq.rearrange("(kk p) m -> kk p m", p=128)
                # interleave per-kk so kk=0 tiles of both land first
                for kk in range(NKK):
                    nc.scalar.dma_start(out=wkv_sb[:, kk, :], in_=wkvr[kk])
                    nc.scalar.dma_start(out=wq_sb[:, kk, :], in_=wqr[kk])
                nc.scalar.dma_start(out=bkv_sb, in_=bkv.rearrange("m p -> p m"))
                nc.scalar.dma_start(out=bq_sb, in_=bq.rearrange("m p -> p m"))
                nc.scalar.dma_start(out=bk_sb, in_=bk.rearrange("m p -> p m"))
                nc.scalar.dma_start(
                    out=wk_sb, in_=wk.rearrange("p (lk m) -> p lk m", lk=NLK))
                nc.scalar.dma_start(
                    out=wv_sb, in_=wv.rearrange("p (lk m) -> p lk m", lk=NLK))
                if causal:
                    nc.scalar.dma_start(out=mb_sb, in_=maskb)

                # PE warmup: dep-light matmuls during the initial DMA wait so
                # the HAM clock gate opens before the real work arrives
                wup = p1p.tile([128, 512], F32, tag="p1p", name="warmup")
                for _ in range(18):
                    nc.tensor.matmul(wup[:], ones1h[0:1, :], warm_mv[0:1, :],
                                     start=True, stop=True)

                for sc in range(NJ):
                    lps = [p1p.tile([128, 512], F32, tag="p1p", name=f"lp{m}")
                           for m in range(NLK)]
                    qps = [p1p.tile([128, 512], F32, tag="p1p", name=f"qp{m}")
                           for m in range(HPC)]
                    for kk in range(NKK):
                        xt = p1x.tile([128, 512], BF16, tag="xt")
                        nc.sync.dma_start(
                            out=xt,
                            in_=xTr[kk, :, sc * 512:(sc + 1) * 512])
                        for m in range(NLK):
                            nc.tensor.matmul(
                                lps[m][:],
                                wkv_sb[:, kk, m * 128:(m + 1) * 128],
                                xt[:],
                                start=(kk == 0), stop=(kk == NKK - 1))
                        for m in range(HPC):
                            nc.tensor.matmul(
                                qps[m][:],
                                wq_sb[:, kk, m * 128:(m + 1) * 128],
                                xt[:],
                                start=(kk == 0), stop=(kk == NKK - 1))
                    # drain lat banks on ACT, qt banks on DVE, in parallel
                    for m in range(NLK):
                        nc.scalar.activation(
                            lat_sb[:, m, sc * 512:(sc + 1) * 512], lps[m][:],
                            mybir.ActivationFunctionType.Identity,
                            bias=bkv_sb[:, m:m + 1])
                    for m in range(HPC):
                        nc.vector.tensor_scalar_add(
                            qt_sb[:, m, sc * 512:(sc + 1) * 512], qps[m][:],
                            bq_sb[:, m:m + 1])

            # wo prefetch deferred past phase 1 so its 4MB doesn't contend
            # with the xt stream for HBM bandwidth; needed first at ~rnd1.
            nc.scalar.dma_start(
                out=wo_sb,
                in_=wo.rearrange("p (h m) -> p h m", h=HPC).bitcast(F32R))

            # ---- phases 2+3+4, interleaved per round (causal) ----
            # Round j: K/V for seq chunk j (bf16), then attention h-groups of
            # q-chunk j with the previous chunk's output projection spliced in
            # so the PE has work during the ACT/DVE-bound softmax stretches.
            # PSUM rings: st(4) + ot(2) + aux(2 — shared by kp/vp/yp) = 8.
            p34 = ctx.enter_context(tc.tile_pool(name="p34", bufs=1))
            ots_sb = p34.tile([128, HPC, S], F32R, tag="ots")
            with tc.tile_pool(name="p3st", bufs=4, space="PSUM") as p3st, \
                 tc.tile_pool(name="paux", bufs=2, space="PSUM") as paux, \
                 tc.tile_pool(name="p3ot", bufs=2, space="PSUM") as p3ot, \
                 tc.tile_pool(name="p3et", bufs=4) as p3et, \
                 tc.tile_pool(name="p3ac", bufs=2) as p3ac, \
                 tc.tile_pool(name="p3sb", bufs=2) as p3sb, \
                 tc.tile_pool(name="p3mt", bufs=2) as p3mt, \
                 tc.tile_pool(name="p4sb", bufs=3) as p4sb:
                def emit_k_group(sc, dm):
                    kp = paux.tile([128, 512], F32, tag="aux",
                                   name=f"kp{sc}_{dm}")
                    for lk in range(NLK):
                        nc.tensor.matmul(
                            kp[:],
                            wk_sb[:, lk, dm * 128:(dm + 1) * 128],
                            lat_sb[:, lk, sc * 512:(sc + 1) * 512],
                            start=(lk == 0), stop=(lk == NLK - 1))
                    nc.vector.tensor_scalar_add(
                        kt_sb[:, dm, sc * 512:(sc + 1) * 512], kp[:],
                        bk_sb[:, dm:dm + 1])

                def emit_v_group(sc, ti):
                    t = sc * 4 + ti
                    vp = paux.tile([128, 512], F32, tag="aux", name=f"vp{t}")
                    for lk in range(NLK):
                        nc.tensor.matmul(
                            vp[:],
                            lat_sb[:, lk, t * 128:(t + 1) * 128],
                            wv_sb[:, lk, :],
                            start=(lk == 0), stop=(lk == NLK - 1))
                    nc.scalar.copy(v_sb[:, t, :], vp[:])

                def emit_ph4_group(jj, gi):
                    # group gi in 0..15 of output chunk jj: 4 head matmuls
                    t = 4 * jj + gi // NJ
                    yc = gi % NJ
                    yp = paux.tile([128, 512], F32, tag="aux",
                                   name=f"yp{jj}_{gi}")
                    for h in range(HPC):
                        nc.tensor.matmul(
                            yp[:],
                            ots_sb[:, h, t * 128:(t + 1) * 128],
                            wo_sb[:, h, yc * 512:(yc + 1) * 512],
                            start=(h == 0), stop=(h == HPC - 1))
                    ys = p4sb.tile([128, 512], BF16, tag="ys")
                    if gi % 2 == 0:
                        nc.scalar.copy(ys[:], yp[:])
                    else:
                        nc.vector.tensor_copy(ys[:], yp[:])
                    nc.sync.dma_start(
                        out=y[t * 128:(t + 1) * 128,
                              yc * 512:(yc + 1) * 512],
                        in_=ys[:])

                def emit_ph3_group(j, h, n_i2, fillers=()):
                    # fillers: list of thunks emitting PE-heavy work, spread
                    # through the i2 loop to cover the ACT-bound exp cadence
                    fillers = list(fillers)
                    every = max(1, n_i2 // max(1, len(fillers))) if fillers else 0
                    ot = p3ot.tile([128, 512], F32, tag="ot")
                    acc = p3ac.tile([128, 512], FP16, tag="acc")
                    for i2 in range(n_i2):
                        if fillers and every and i2 % every == every - 1:
                            fillers.pop(0)()
                        r = i2 - 4 * j if causal else -1
                        off = 128 * r if r > 0 else 0
                        st = p3st.tile([128, 512], F32, tag="st")
                        nc.tensor.matmul(
                            st[:, off:512],
                            kt_sb[:, h, i2 * 128:(i2 + 1) * 128],
                            qt_sb[:, h, j * 512 + off:(j + 1) * 512],
                            start=True, stop=True)
                        if causal:
                            if r >= 0:
                                nc.vector.tensor_add(
                                    st[:, off:off + 128],
                                    st[:, off:off + 128], mb_sb[:])
                        else:
                            mt = p3mt.tile([128, 512], F32, tag="mt")
                            nc.sync.dma_start(
                                out=mt,
                                in_=maskb.rearrange(
                                    "(i p) q -> i p q", p=128)
                                [i2, :, j * 512:(j + 1) * 512])
                            nc.vector.tensor_add(st[:], st[:], mt[:])
                        et = p3et.tile([128, 512], FP16, tag="et")
                        nc.scalar.activation(et[:, off:512], st[:, off:512],
                                             EXP, scale=SCALE)
                        if i2 == 0:
                            nc.vector.tensor_copy(acc[:], et[:])
                        else:
                            nc.vector.tensor_add(
                                acc[:, off:512], acc[:, off:512],
                                et[:, off:512])
                        nc.tensor.matmul(
                            ot[:, off:512],
                            v_sb[:, i2, h * 128:(h + 1) * 128],
                            et[:, off:512],
                            start=(i2 == 0), stop=(i2 == n_i2 - 1))
                    cs = p3st.tile([128, 512], F32, tag="st", name="cs_st")
                    nc.tensor.matmul(cs[0:1, :], oneskh[:, 0:1],
                                     acc[:], start=True, stop=True)
                    csb = p3sb.tile([1, 512], FP16, tag="csb")
                    nc.vector.tensor_copy(csb[0:1, :], cs[0:1, :])
                    rb = p3st.tile([128, 512], F32, tag="st", name="rb_st")
                    nc.tensor.matmul(rb[:], ones1h[0:1, :],
                                     csb[0:1, :], start=True, stop=True)
                    rs = p3sb.tile([128, 512], F32, tag="rs")
                    nc.vector.reciprocal_approx_fast(out=rs[:], in_=rb[:])
                    nc.vector.tensor_mul(
                        ots_sb[:, h, j * 512:(j + 1) * 512], ot[:], rs[:])
                    for f in fillers:
                        f()

                if causal:
                    for rnd in range(NJ):
                        for ti in range(4):
                            emit_v_group(rnd, ti)
                        for h in range(HPC):
                            # K for head h of this round, plus the previous
                            # chunk's output-projection groups as PE filler.
                            # At rnd 0 every k-tile is diagonal, so K must be
                            # emitted before the ph3 group (PE queue order).
                            if rnd == 0:
                                emit_k_group(rnd, h)
                                fill = []
                            else:
                                fill = [lambda d=h, r=rnd: emit_k_group(r, d)]
                                fill += [
                                    lambda g=gi, r=rnd: emit_ph4_group(r - 1, g)
                                    for gi in range(4 * h, 4 * h + 4)]
                            emit_ph3_group(rnd, h, 4 * rnd + 4, fill)
                    for gi in range(16):
                        emit_ph4_group(NJ - 1, gi)
                else:
                    for sc in range(NJ):
                        for ti in range(4):
                            emit_v_group(sc, ti)
                        for dm in range(HPC):
                            emit_k_group(sc, dm)
                    for j in range(NJ):
                        for h in range(HPC):
                            emit_ph3_group(j, h, NQT)
                        for gi in range(16):
                            emit_ph4_group(j, gi)

    nc.compile()
    _BUILD_CACHE[causal] = nc
    return nc


def kernel(**inputs) -> np.ndarray:
    x = np.asarray(inputs["x"], dtype=np.float32)
    mask = np.asarray(inputs["mask"])
    Wq = np.asarray(inputs["Wq"], dtype=np.float32)
    bq = np.asarray(inputs["bq"], dtype=np.float32)
    Wkv = np.asarray(inputs["Wkv"], dtype=np.float32)
    bkv = np.asarray(inputs["bkv"], dtype=np.float32)
    Wk = np.asarray(inputs["Wk"], dtype=np.float32)
    bk = np.asarray(inputs["bk"], dtype=np.float32)
    Wv = np.asarray(inputs["Wv"], dtype=np.float32)
    bv = np.asarray(inputs["bv"], dtype=np.float32)
    Wo = np.asarray(inputs["Wo"], dtype=np.float32)
    bo = np.asarray(inputs["bo"], dtype=np.float32)

    tril = np.tril(np.ones((S, S), dtype=mask.dtype))
    causal = all(np.array_equal(mask[b], tril) for b in range(B))
    nc = build(causal)

    bf = lambda a: np.ascontiguousarray(a).astype(ml_dtypes.bfloat16)

    def wt(a, dt=None):
        # [K, M] -> [128, (K//128)*M]: contraction-tile-major, partition-first
        k, mm_ = a.shape
        out = np.ascontiguousarray(
            a.reshape(k // 128, 128, mm_).transpose(1, 0, 2).reshape(128, -1))
        return out.astype(dt) if dt is not None else bf(out)

    # triangular NEG bias for the 128-wide diagonal block: mask where f < p
    if causal:
        p = np.arange(128)[:, None]
        f = np.arange(128)[None, :]
        mb = np.where(f < p, NEG, 0.0).astype(np.float32)

    in_maps = []
    for c in range(N_CORES):
        b, g = divmod(c, HG)
        sl = slice(g * HSL, (g + 1) * HSL)
        m = {
            "xT": bf(x[b].T),
            "wq": bf(Wq[:, sl]),
            "bq": np.ascontiguousarray(bq[sl]).reshape(HPC, 128),
            "wkv": bf(Wkv),
            "bkv": bkv.reshape(NLK, 128),
            "wk": wt(Wk[:, sl]),
            "bk": np.ascontiguousarray(bk[sl]).reshape(HPC, 128),
            "wv": wt(Wv[:, sl]),
            "wo": wt(Wo[sl, :], np.float32),
        }
        if causal:
            m["maskb"] = mb
        else:
            m["maskb"] = np.ascontiguousarray(
                np.where(mask[b] == 0, NEG, 0.0).astype(np.float32))
        in_maps.append(m)

    res = run_bass_kernel_spmd(nc, in_maps, list(range(N_CORES)))
    bo_eff = (bo + bv @ Wo).astype(np.float32)
    out = np.empty((B, S, D_MODEL), dtype=np.float32)
    for b in range(B):
        acc = res.results[b * HG]["y"].astype(np.float32).copy()
        for g in range(1, HG):
            acc += res.results[b * HG + g]["y"]
        out[b] = acc + bo_eff
    return out


# revision 46
# speedup vs baseline: 1.0210x; 1.0210x over previous
"""MultiHeadLatentAttention Trainium2 kernel (8 NeuronCores, SPMD).

Sharding: batch (2) x head-group (4 of 4 heads each) -> 8 cores.
Each core computes, for its batch b and heads [4g, 4g+4):
  phase 1 (merged, bf16): latT = Wkv^T x^T + bkv AND QT = Wq_sl^T x^T + bq
    in one pass over x^T tiles (8 PSUM banks: 4 lat + 4 q).
  phase 2 (bf16): KT = Wk_sl^T latT + bk ; V = latT^T Wv_sl (no bias --
    A rows sum to 1 so bv folds into the host-side bo via bo + bv @ Wo).
  phase 3 (f32r): per head h, q-chunk j (512 wide), k-tile i2:
    causal trim: diagonal k-tile r=i2-4j only covers q in [128r, 512).
    ST[k,q] = KT_h[:,i2]^T QT_h[:,j]  on the live region
    diag block masked via one [128,128] triangular NEG add (DVE)
    ET = exp(scale*ST) (ACT); ET tiles presummed on DVE into acc;
    OT[dh,q] += V_h[i2]^T ET ; then ONE colsum matmul ones^T acc,
    reciprocal + rank-1 broadcast, OTs = OT * (1/colsum).
  phase 4 (f32r): y_partial[q,:] = sum_h OTs_h[:,qtile]^T Wo_sl_h;
    PSUM->SBUF copy on ACT, DMA out on sync.
Host: y[b] = sum of 4 partials + (bo + bv @ Wo).

Matmul dtype rule (empirical): the MOVING operand of an f32r matmul must be
produced by instructions writing f32r-typed APs; the stationary operand can
be f32-written and bitcast at the matmul. bf16 ops have no such rule.
"""
import sys

sys.path.insert(0, "/opt/trn_rl_repo")

import math
import numpy as np
import ml_dtypes

import concourse.bass as bass
import concourse.mybir as mybir
import concourse.tile as tile
from concourse import bacc
from concourse.bass_utils import run_bass_kernel_spmd

F32 = mybir.dt.float32
F32R = mybir.dt.float32r
BF16 = mybir.dt.bfloat16
FP16 = mybir.dt.float16
EXP = mybir.ActivationFunctionType.Exp

D_MODEL = 2048
NUM_HEADS = 16
D_HEAD = 128
D_LATENT = 512
B, S = 2, 2048
N_CORES = 8
HG = 4                      # head-groups (cores per batch)
HPC = NUM_HEADS // HG       # heads per core = 4
HSL = HPC * D_HEAD          # head-dim slice per core = 512
NQT = S // 128              # 16 q tiles of 128
NJ = S // 512               # 4 q chunks of 512
NKK = D_MODEL // 128        # 16 contraction chunks
NLK = D_LATENT // 128       # 4 latent chunks
SCALE = 1.0 / math.sqrt(D_HEAD)
NEG = -1.0e9

_BUILD_CACHE = {}


def build(causal: bool):
    if causal in _BUILD_CACHE:
        return _BUILD_CACHE[causal]
    nc = bacc.Bacc("TRN2", target_bir_lowering=False, debug=False,
                   num_devices=N_CORES)

    # weights arrive host-pretransposed to [128 partitions, ...] so each
    # lands in one contiguous large-line DMA
    xT = nc.dram_tensor("xT", [D_MODEL, S], BF16, kind="ExternalInput").ap()
    wq = nc.dram_tensor("wq", [D_MODEL, HSL], BF16, kind="ExternalInput").ap()
    bq = nc.dram_tensor("bq", [HPC, 128], F32, kind="ExternalInput").ap()
    wkv = nc.dram_tensor("wkv", [D_MODEL, D_LATENT], BF16,
                         kind="ExternalInput").ap()
    bkv = nc.dram_tensor("bkv", [NLK, 128], F32, kind="ExternalInput").ap()
    wk = nc.dram_tensor("wk", [128, NLK * HSL], BF16, kind="ExternalInput").ap()
    bk = nc.dram_tensor("bk", [HPC, 128], F32, kind="ExternalInput").ap()
    wv = nc.dram_tensor("wv", [128, NLK * HSL], BF16, kind="ExternalInput").ap()
    wo = nc.dram_tensor("wo", [128, HPC * D_MODEL], F32, kind="ExternalInput").ap()
    if causal:
        maskb = nc.dram_tensor("maskb", [128, 128], F32,
                               kind="ExternalInput").ap()
    else:
        maskb = nc.dram_tensor("maskb", [S, S], F32, kind="ExternalInput").ap()
    y = nc.dram_tensor("y", [S, D_MODEL], BF16, kind="ExternalOutput").ap()

    xTr = xT.rearrange("(kk p) s -> kk p s", p=128)

    with tile.TileContext(nc) as tc:
        from contextlib import ExitStack
        with ExitStack() as ctx:
            # ---- pools, strictly LIFO lifetimes ----
            consts = ctx.enter_context(tc.tile_pool(name="consts", bufs=1))
            persist = ctx.enter_context(tc.tile_pool(name="persist", bufs=1))
            qt_sb = persist.tile([128, HPC, S], F32R, tag="qt")
            kv_pool = ctx.enter_context(tc.tile_pool(name="kvp", bufs=1))
            kt_sb = kv_pool.tile([128, HPC, S], F32R, tag="kt")
            v_sb = kv_pool.tile([128, NQT, HSL], FP16, tag="v")
            wk_sb = kv_pool.tile([128, NLK, HSL], BF16, tag="wk")
            wv_sb = kv_pool.tile([128, NLK, HSL], BF16, tag="wv")
            wo_sb = kv_pool.tile([128, HPC, D_MODEL], F32R, tag="wo")

            ones1 = consts.tile([1, 128], F32, tag="ones1")
            onesk = consts.tile([128, 1], F32, tag="onesk")
            ones1h = consts.tile([1, 128], FP16, tag="ones1h")
            oneskh = consts.tile([128, 1], FP16, tag="oneskh")
            bq_sb = consts.tile([128, HPC], F32, tag="bq")
            bkv_sb = consts.tile([128, NLK], F32, tag="bkv")
            bk_sb = consts.tile([128, HPC], F32, tag="bk")
            if causal:
                mb_sb = consts.tile([128, 128], F32, tag="mb")

            warm_mv = consts.tile([1, 512], FP16, tag="warm")
            nc.vector.memset(ones1, 1.0)
            nc.vector.memset(onesk, 1.0)
            nc.vector.memset(ones1h, 1.0)
            nc.vector.memset(oneskh, 1.0)
            nc.vector.memset(warm_mv, 1.0)

            lat_sb = kv_pool.tile([128, NLK, S], BF16, tag="lat")

            # ---- phase 1 (merged): latT = Wkv^T xT + bkv ; QT = Wq^T xT + bq
            with tc.tile_pool(name="p1w", bufs=1) as p1w, \
                 tc.tile_pool(name="p1x", bufs=10) as p1x, \
                 tc.tile_pool(name="p1p", bufs=8, space="PSUM") as p1p:
                wkv_sb = p1w.tile([128, NKK, D_LATENT], BF16, tag="wkv")
                wq_sb = p1w.tile([128, NKK, HSL], BF16, tag="wq")
                wkvr = wkv.rearrange("(kk p) m -> kk p m", p=128)
                wqr = wq.rearrange("(kk p) m -> kk p m", p=128)
                # interleave per-kk so kk=0 tiles of both land first
                for kk in range(NKK):
                    nc.scalar.dma_start(out=wkv_sb[:, kk, :], in_=wkvr[kk])
                    nc.scalar.dma_start(out=wq_sb[:, kk, :], in_=wqr[kk])
                nc.scalar.dma_start(out=bkv_sb, in_=bkv.rearrange("m p -> p m"))
                nc.scalar.dma_start(out=bq_sb, in_=bq.rearrange("m p -> p m"))
                nc.scalar.dma_start(out=bk_sb, in_=bk.rearrange("m p -> p m"))
                nc.scalar.dma_start(
                    out=wk_sb, in_=wk.rearrange("p (lk m) -> p lk m", lk=NLK))
                nc.scalar.dma_start(
                    out=wv_sb, in_=wv.rearrange("p (lk m) -> p lk m", lk=NLK))
                if causal:
                    nc.scalar.dma_start(out=mb_sb, in_=maskb)

                for sc in range(NJ):
                    lps = [p1p.tile([128, 512], F32, tag="p1p", name=f"lp{m}")
                           for m in range(NLK)]
                    qps = [p1p.tile([128, 512], F32, tag="p1p", name=f"qp{m}")
                           for m in range(HPC)]
                    for kk in range(NKK):
                        xt = p1x.tile([128, 512], BF16, tag="xt")
                        nc.sync.dma_start(
                            out=xt,
                            in_=xTr[kk, :, sc * 512:(sc + 1) * 512])
                        for m in range(NLK):
                            nc.tensor.matmul(
                                lps[m][:],
                                wkv_sb[:, kk, m * 128:(m + 1) * 128],
                                xt[:],
                                start=(kk == 0), stop=(kk == NKK - 1))
                        for m in range(HPC):
                            nc.tensor.matmul(
                                qps[m][:],
                                wq_sb[:, kk, m * 128:(m + 1) * 128],
                                xt[:],
                                start=(kk == 0), stop=(kk == NKK - 1))
                    # drain lat banks on ACT, qt banks on DVE, in parallel
                    for m in range(NLK):
                        nc.scalar.activation(
                            lat_sb[:, m, sc * 512:(sc + 1) * 512], lps[m][:],
                            mybir.ActivationFunctionType.Identity,
                            bias=bkv_sb[:, m:m + 1])
                    for m in range(HPC):
                        nc.vector.tensor_scalar_add(
                            qt_sb[:, m, sc * 512:(sc + 1) * 512], qps[m][:],
                            bq_sb[:, m:m + 1])

            # wo prefetch deferred past phase 1 so its 4MB doesn't contend
            # with the xt stream for HBM bandwidth; needed first at ~rnd1.
            nc.scalar.dma_start(
                out=wo_sb,
                in_=wo.rearrange("p (h m) -> p h m", h=HPC).bitcast(F32R))

            # ---- phases 2+3+4, interleaved per round (causal) ----
            # Round j: K/V for seq chunk j (bf16), then attention h-groups of
            # q-chunk j with the previous chunk's output projection spliced in
            # so the PE has work during the ACT/DVE-bound softmax stretches.
            # PSUM rings: st(4) + ot(2) + aux(2 — shared by kp/vp/yp) = 8.
            p34 = ctx.enter_context(tc.tile_pool(name="p34", bufs=1))
            ots_sb = p34.tile([128, HPC, S], F32R, tag="ots")
            with tc.tile_pool(name="p3st", bufs=4, space="PSUM") as p3st, \
                 tc.tile_pool(name="paux", bufs=2, space="PSUM") as paux, \
                 tc.tile_pool(name="p3ot", bufs=2, space="PSUM") as p3ot, \
                 tc.tile_pool(name="p3et", bufs=4) as p3et, \
                 tc.tile_pool(name="p3ac", bufs=2) as p3ac, \
                 tc.tile_pool(name="p3sb", bufs=2) as p3sb, \
                 tc.tile_pool(name="p3mt", bufs=2) as p3mt, \
                 tc.tile_pool(name="p4sb", bufs=3) as p4sb:
                def emit_k_group(sc, dm):
                    kp = paux.tile([128, 512], F32, tag="aux",
                                   name=f"kp{sc}_{dm}")
                    for lk in range(NLK):
                        nc.tensor.matmul(
                            kp[:],
                            wk_sb[:, lk, dm * 128:(dm + 1) * 128],
                            lat_sb[:, lk, sc * 512:(sc + 1) * 512],
                            start=(lk == 0), stop=(lk == NLK - 1))
                    nc.vector.tensor_scalar_add(
                        kt_sb[:, dm, sc * 512:(sc + 1) * 512], kp[:],
                        bk_sb[:, dm:dm + 1])

                def emit_v_group(sc, ti):
                    t = sc * 4 + ti
                    vp = paux.tile([128, 512], F32, tag="aux", name=f"vp{t}")
                    for lk in range(NLK):
                        nc.tensor.matmul(
                            vp[:],
                            lat_sb[:, lk, t * 128:(t + 1) * 128],
                            wv_sb[:, lk, :],
                            start=(lk == 0), stop=(lk == NLK - 1))
                    nc.scalar.copy(v_sb[:, t, :], vp[:])

                def emit_ph4_group(jj, gi):
                    # group gi in 0..15 of output chunk jj: 4 head matmuls
                    t = 4 * jj + gi // NJ
                    yc = gi % NJ
                    yp = paux.tile([128, 512], F32, tag="aux",
                                   name=f"yp{jj}_{gi}")
                    for h in range(HPC):
                        nc.tensor.matmul(
                            yp[:],
                            ots_sb[:, h, t * 128:(t + 1) * 128],
                            wo_sb[:, h, yc * 512:(yc + 1) * 512],
                            start=(h == 0), stop=(h == HPC - 1))
                    ys = p4sb.tile([128, 512], BF16, tag="ys")
                    if gi % 2 == 0:
                        nc.scalar.copy(ys[:], yp[:])
                    else:
                        nc.vector.tensor_copy(ys[:], yp[:])
                    nc.sync.dma_start(
                        out=y[t * 128:(t + 1) * 128,
                              yc * 512:(yc + 1) * 512],
                        in_=ys[:])

                def emit_ph3_group(j, h, n_i2, fillers=()):
                    # fillers: list of thunks emitting PE-heavy work, spread
                    # through the i2 loop to cover the ACT-bound exp cadence
                    fillers = list(fillers)
                    every = max(1, n_i2 // max(1, len(fillers))) if fillers else 0
                    ot = p3ot.tile([128, 512], F32, tag="ot")
                    acc = p3ac.tile([128, 512], FP16, tag="acc")
                    for i2 in range(n_i2):
                        if fillers and every and i2 % every == every - 1:
                            fillers.pop(0)()
                        r = i2 - 4 * j if causal else -1
                        off = 128 * r if r > 0 else 0
                        st = p3st.tile([128, 512], F32, tag="st")
                        nc.tensor.matmul(
                            st[:, off:512],
                            kt_sb[:, h, i2 * 128:(i2 + 1) * 128],
                            qt_sb[:, h, j * 512 + off:(j + 1) * 512],
                            start=True, stop=True)
                        if causal:
                            if r >= 0:
                                nc.vector.tensor_add(
                                    st[:, off:off + 128],
                                    st[:, off:off + 128], mb_sb[:])
                        else:
                            mt = p3mt.tile([128, 512], F32, tag="mt")
                            nc.sync.dma_start(
                                out=mt,
                                in_=maskb.rearrange(
                                    "(i p) q -> i p q", p=128)
                                [i2, :, j * 512:(j + 1) * 512])
                            nc.vector.tensor_add(st[:], st[:], mt[:])
                        et = p3et.tile([128, 512], FP16, tag="et")
                        nc.scalar.activation(et[:, off:512], st[:, off:512],
                                             EXP, scale=SCALE)
                        if i2 == 0:
                            nc.vector.tensor_copy(acc[:], et[:])
                        else:
                            nc.vector.tensor_add(
                                acc[:, off:512], acc[:, off:512],
                                et[:, off:512])
                        nc.tensor.matmul(
                            ot[:, off:512],
                            v_sb[:, i2, h * 128:(h + 1) * 128],
                            et[:, off:512],
                            start=(i2 == 0), stop=(i2 == n_i2 - 1))
                    cs = p3st.tile([128, 512], F32, tag="st", name="cs_st")
                    nc.tensor.matmul(cs[0:1, :], oneskh[:, 0:1],
                                     acc[:], start=True, stop=True)
                    csb = p3sb.tile([1, 512], FP16, tag="csb")
                    nc.vector.tensor_copy(csb[0:1, :], cs[0:1, :])
                    rb = p3st.tile([128, 512], F32, tag="st", name="rb_st")
                    nc.tensor.matmul(rb[:], ones1h[0:1, :],
                                     csb[0:1, :], start=True, stop=True)
                    rs = p3sb.tile([128, 512], F32, tag="rs")
                    nc.vector.reciprocal_approx_fast(out=rs[:], in_=rb[:])
                    nc.vector.tensor_mul(
                        ots_sb[:, h, j * 512:(j + 1) * 512], ot[:], rs[:])
                    for f in fillers:
                        f()

                if causal:
                    for rnd in range(NJ):
                        for ti in range(4):
                            emit_v_group(rnd, ti)
                        for h in range(HPC):
                            # K for head h of this round, plus the previous
                            # chunk's output-projection groups as PE filler.
                            # At rnd 0 every k-tile is diagonal, so K must be
                            # emitted before the ph3 group (PE queue order).
                            if rnd == 0:
                                emit_k_group(rnd, h)
                                fill = []
                            else:
                                fill = [lambda d=h, r=rnd: emit_k_group(r, d)]
                                fill += [
                                    lambda g=gi, r=rnd: emit_ph4_group(r - 1, g)
                                    for gi in range(4 * h, 4 * h + 4)]
                            emit_ph3_group(rnd, h, 4 * rnd + 4, fill)
                    for gi in range(16):
                        emit_ph4_group(NJ - 1, gi)
                else:
                    for sc in range(NJ):
                        for ti in range(4):
                            emit_v_group(sc, ti)
                        for dm in range(HPC):
                            emit_k_group(sc, dm)
                    for j in range(NJ):
                        for h in range(HPC):
                            emit_ph3_group(j, h, NQT)
                        for gi in range(16):
                            emit_ph4_group(j, gi)

    nc.compile()
    _BUILD_CACHE[causal] = nc
    return nc


def kernel(**inputs) -> np.ndarray:
    x = np.asarray(inputs["x"], dtype=np.float32)
    mask = np.asarray(inputs["mask"])
    Wq = np.asarray(inputs["Wq"], dtype=np.float32)
    bq = np.asarray(inputs["bq"], dtype=np.float32)
    Wkv = np.asarray(inputs["Wkv"], dtype=np.float32)
    bkv = np.asarray(inputs["bkv"], dtype=np.float32)
    Wk = np.asarray(inputs["Wk"], dtype=np.float32)
    bk = np.asarray(inputs["bk"], dtype=np.float32)
    Wv = np.asarray(inputs["Wv"], dtype=np.float32)
    bv = np.asarray(inputs["bv"], dtype=np.float32)
    Wo = np.asarray(inputs["Wo"], dtype=np.float32)
    bo = np.asarray(inputs["bo"], dtype=np.float32)

    tril = np.tril(np.ones((S, S), dtype=mask.dtype))
    causal = all(np.array_equal(mask[b], tril) for b in range(B))
    nc = build(causal)

    bf = lambda a: np.ascontiguousarray(a).astype(ml_dtypes.bfloat16)

    def wt(a, dt=None):
        # [K, M] -> [128, (K//128)*M]: contraction-tile-major, partition-first
        k, mm_ = a.shape
        out = np.ascontiguousarray(
            a.reshape(k // 128, 128, mm_).transpose(1, 0, 2).reshape(128, -1))
        return out.astype(dt) if dt is not None else bf(out)

    # triangular NEG bias for the 128-wide diagonal block: mask where f < p
    if causal:
        p = np.arange(128)[:, None]
        f = np.arange(128)[None, :]
        mb = np.where(f < p, NEG, 0.0).astype(np.float32)

    in_maps = []
    for c in range(N_CORES):
        b, g = divmod(c, HG)
        sl = slice(g * HSL, (g + 1) * HSL)
        m = {
            "xT": bf(x[b].T),
            "wq": bf(Wq[:, sl]),
            "bq": np.ascontiguousarray(bq[sl]).reshape(HPC, 128),
            "wkv": bf(Wkv),
            "bkv": bkv.reshape(NLK, 128),
            "wk": wt(Wk[:, sl]),
            "bk": np.ascontiguousarray(bk[sl]).reshape(HPC, 128),
            "wv": wt(Wv[:, sl]),
            "wo": wt(Wo[sl, :], np.float32),
        }
        if causal:
            m["maskb"] = mb
        else:
            m["maskb"] = np.ascontiguousarray(
                np.where(mask[b] == 0, NEG, 0.0).astype(np.float32))
        in_maps.append(m)

    res = run_bass_kernel_spmd(nc, in_maps, list(range(N_CORES)))
    bo_eff = (bo + bv @ Wo).astype(np.float32)
    out = np.empty((B, S, D_MODEL), dtype=np.float32)
    for b in range(B):
        acc = res.results[b * HG]["y"].astype(np.float32).copy()
        for g in range(1, HG):
            acc += res.results[b * HG + g]["y"]
        out[b] = acc + bo_eff
    return out


# revision 47
# speedup vs baseline: 1.0582x; 1.0364x over previous
"""MultiHeadLatentAttention Trainium2 kernel (8 NeuronCores, SPMD).

Sharding: batch (2) x head-group (4 of 4 heads each) -> 8 cores.
Each core computes, for its batch b and heads [4g, 4g+4):
  phase 1 (merged, bf16): latT = Wkv^T x^T + bkv AND QT = Wq_sl^T x^T + bq
    in one pass over x^T tiles (8 PSUM banks: 4 lat + 4 q).
  phase 2 (bf16): KT = Wk_sl^T latT + bk ; V = latT^T Wv_sl (no bias --
    A rows sum to 1 so bv folds into the host-side bo via bo + bv @ Wo).
  phase 3 (f32r): per head h, q-chunk j (512 wide), k-tile i2:
    causal trim: diagonal k-tile r=i2-4j only covers q in [128r, 512).
    ST[k,q] = KT_h[:,i2]^T QT_h[:,j]  on the live region
    diag block masked via one [128,128] triangular NEG add (DVE)
    ET = exp(scale*ST) (ACT); ET tiles presummed on DVE into acc;
    OT[dh,q] += V_h[i2]^T ET ; then ONE colsum matmul ones^T acc,
    reciprocal + rank-1 broadcast, OTs = OT * (1/colsum).
  phase 4 (f32r): y_partial[q,:] = sum_h OTs_h[:,qtile]^T Wo_sl_h;
    PSUM->SBUF copy on ACT, DMA out on sync.
Host: y[b] = sum of 4 partials + (bo + bv @ Wo).

Matmul dtype rule (empirical): the MOVING operand of an f32r matmul must be
produced by instructions writing f32r-typed APs; the stationary operand can
be f32-written and bitcast at the matmul. bf16 ops have no such rule.
"""
import sys

sys.path.insert(0, "/opt/trn_rl_repo")

import math
import numpy as np
import ml_dtypes

import concourse.bass as bass
import concourse.mybir as mybir
import concourse.tile as tile
from concourse import bacc
from concourse.bass_utils import run_bass_kernel_spmd

F32 = mybir.dt.float32
F32R = mybir.dt.float32r
BF16 = mybir.dt.bfloat16
FP16 = mybir.dt.float16
EXP = mybir.ActivationFunctionType.Exp

D_MODEL = 2048
NUM_HEADS = 16
D_HEAD = 128
D_LATENT = 512
B, S = 2, 2048
N_CORES = 8
HG = 4                      # head-groups (cores per batch)
HPC = NUM_HEADS // HG       # heads per core = 4
HSL = HPC * D_HEAD          # head-dim slice per core = 512
NQT = S // 128              # 16 q tiles of 128
NJ = S // 512               # 4 q chunks of 512
NKK = D_MODEL // 128        # 16 contraction chunks
NLK = D_LATENT // 128       # 4 latent chunks
SCALE = 1.0 / math.sqrt(D_HEAD)
NEG = -1.0e9

_BUILD_CACHE = {}


def build(causal: bool):
    if causal in _BUILD_CACHE:
        return _BUILD_CACHE[causal]
    nc = bacc.Bacc("TRN2", target_bir_lowering=False, debug=False,
                   num_devices=N_CORES)

    # weights arrive host-pretransposed to [128 partitions, ...] so each
    # lands in one contiguous large-line DMA
    xT = nc.dram_tensor("xT", [D_MODEL, S], BF16, kind="ExternalInput").ap()
    wq = nc.dram_tensor("wq", [D_MODEL, HSL], BF16, kind="ExternalInput").ap()
    bq = nc.dram_tensor("bq", [HPC, 128], F32, kind="ExternalInput").ap()
    wkv = nc.dram_tensor("wkv", [D_MODEL, D_LATENT], BF16,
                         kind="ExternalInput").ap()
    bkv = nc.dram_tensor("bkv", [NLK, 128], F32, kind="ExternalInput").ap()
    wk = nc.dram_tensor("wk", [128, NLK * HSL], BF16, kind="ExternalInput").ap()
    bk = nc.dram_tensor("bk", [HPC, 128], F32, kind="ExternalInput").ap()
    wv = nc.dram_tensor("wv", [128, NLK * HSL], BF16, kind="ExternalInput").ap()
    wo = nc.dram_tensor("wo", [128, HPC * D_MODEL], F32, kind="ExternalInput").ap()
    if causal:
        maskb = nc.dram_tensor("maskb", [128, 128], F32,
                               kind="ExternalInput").ap()
    else:
        maskb = nc.dram_tensor("maskb", [S, S], F32, kind="ExternalInput").ap()
    y = nc.dram_tensor("y", [S, D_MODEL], BF16, kind="ExternalOutput").ap()

    xTr = xT.rearrange("(kk p) s -> kk p s", p=128)

    with tile.TileContext(nc) as tc:
        from contextlib import ExitStack
        with ExitStack() as ctx:
            # ---- pools, strictly LIFO lifetimes ----
            consts = ctx.enter_context(tc.tile_pool(name="consts", bufs=1))
            persist = ctx.enter_context(tc.tile_pool(name="persist", bufs=1))
            qt_sb = persist.tile([128, HPC, S], F32R, tag="qt")
            kv_pool = ctx.enter_context(tc.tile_pool(name="kvp", bufs=1))
            kt_sb = kv_pool.tile([128, HPC, S], F32R, tag="kt")
            v_sb = kv_pool.tile([128, NQT, HSL], FP16, tag="v")
            wk_sb = kv_pool.tile([128, NLK, HSL], BF16, tag="wk")
            wv_sb = kv_pool.tile([128, NLK, HSL], BF16, tag="wv")
            wo_sb = kv_pool.tile([128, HPC, D_MODEL], F32R, tag="wo")

            ones1 = consts.tile([1, 128], F32, tag="ones1")
            onesk = consts.tile([128, 1], F32, tag="onesk")
            ones1h = consts.tile([1, 128], FP16, tag="ones1h")
            oneskh = consts.tile([128, 1], FP16, tag="oneskh")
            bq_sb = consts.tile([128, HPC], F32, tag="bq")
            bkv_sb = consts.tile([128, NLK], F32, tag="bkv")
            bk_sb = consts.tile([128, HPC], F32, tag="bk")
            if causal:
                mb_sb = consts.tile([128, 128], F32, tag="mb")

            warm_mv = consts.tile([1, 512], FP16, tag="warm")
            nc.vector.memset(ones1, 1.0)
            nc.vector.memset(onesk, 1.0)
            nc.vector.memset(ones1h, 1.0)
            nc.vector.memset(oneskh, 1.0)
            nc.vector.memset(warm_mv, 1.0)

            lat_sb = kv_pool.tile([128, NLK, S], BF16, tag="lat")

            # ---- phase 1 (merged): latT = Wkv^T xT + bkv ; QT = Wq^T xT + bq
            with tc.tile_pool(name="p1w", bufs=1) as p1w, \
                 tc.tile_pool(name="p1x", bufs=10) as p1x, \
                 tc.tile_pool(name="p1p", bufs=8, space="PSUM") as p1p:
                wkv_sb = p1w.tile([128, NKK, D_LATENT], BF16, tag="wkv")
                wq_sb = p1w.tile([128, NKK, HSL], BF16, tag="wq")
                wkvr = wkv.rearrange("(kk p) m -> kk p m", p=128)
                wqr = wq.rearrange("(kk p) m -> kk p m", p=128)
                # interleave per-kk so kk=0 tiles of both land first
                for kk in range(NKK):
                    nc.scalar.dma_start(out=wkv_sb[:, kk, :], in_=wkvr[kk])
                    nc.scalar.dma_start(out=wq_sb[:, kk, :], in_=wqr[kk])
                nc.scalar.dma_start(out=bkv_sb, in_=bkv.rearrange("m p -> p m"))
                nc.scalar.dma_start(out=bq_sb, in_=bq.rearrange("m p -> p m"))
                nc.scalar.dma_start(out=bk_sb, in_=bk.rearrange("m p -> p m"))
                nc.scalar.dma_start(
                    out=wk_sb, in_=wk.rearrange("p (lk m) -> p lk m", lk=NLK))
                nc.scalar.dma_start(
                    out=wv_sb, in_=wv.rearrange("p (lk m) -> p lk m", lk=NLK))
                if causal:
                    nc.scalar.dma_start(out=mb_sb, in_=maskb)

                for sc in range(NJ):
                    lps = [p1p.tile([128, 512], F32, tag="p1p", name=f"lp{m}")
                           for m in range(NLK)]
                    qps = [p1p.tile([128, 512], F32, tag="p1p", name=f"qp{m}")
                           for m in range(HPC)]
                    for kk in range(NKK):
                        xt = p1x.tile([128, 512], BF16, tag="xt")
                        nc.sync.dma_start(
                            out=xt,
                            in_=xTr[kk, :, sc * 512:(sc + 1) * 512])
                        for m in range(NLK):
                            nc.tensor.matmul(
                                lps[m][:],
                                wkv_sb[:, kk, m * 128:(m + 1) * 128],
                                xt[:],
                                start=(kk == 0), stop=(kk == NKK - 1))
                        for m in range(HPC):
                            nc.tensor.matmul(
                                qps[m][:],
                                wq_sb[:, kk, m * 128:(m + 1) * 128],
                                xt[:],
                                start=(kk == 0), stop=(kk == NKK - 1))
                    # drain lat banks on ACT, qt banks on DVE, in parallel
                    for m in range(NLK):
                        nc.scalar.activation(
                            lat_sb[:, m, sc * 512:(sc + 1) * 512], lps[m][:],
                            mybir.ActivationFunctionType.Identity,
                            bias=bkv_sb[:, m:m + 1])
                    for m in range(HPC):
                        nc.vector.tensor_scalar_add(
                            qt_sb[:, m, sc * 512:(sc + 1) * 512], qps[m][:],
                            bq_sb[:, m:m + 1])

            # wo prefetch deferred past phase 1 so its 4MB doesn't contend
            # with the xt stream for HBM bandwidth; needed first at ~rnd1.
            wor = wo.rearrange("p (h m) -> p h m", h=HPC).bitcast(F32R)
            for h_ in range(HPC):
                nc.scalar.dma_start(out=wo_sb[:, h_, :], in_=wor[:, h_])

            # ---- phases 2+3+4, interleaved per round (causal) ----
            # Round j: K/V for seq chunk j (bf16), then attention h-groups of
            # q-chunk j with the previous chunk's output projection spliced in
            # so the PE has work during the ACT/DVE-bound softmax stretches.
            # PSUM rings: st(4) + ot(2) + aux(2 — shared by kp/vp/yp) = 8.
            p34 = ctx.enter_context(tc.tile_pool(name="p34", bufs=1))
            ots_sb = p34.tile([128, HPC, S], F32R, tag="ots")
            with tc.tile_pool(name="p3st", bufs=4, space="PSUM") as p3st, \
                 tc.tile_pool(name="paux", bufs=2, space="PSUM") as paux, \
                 tc.tile_pool(name="p3ot", bufs=2, space="PSUM") as p3ot, \
                 tc.tile_pool(name="p3et", bufs=4) as p3et, \
                 tc.tile_pool(name="p3ac", bufs=2) as p3ac, \
                 tc.tile_pool(name="p3sb", bufs=2) as p3sb, \
                 tc.tile_pool(name="p3mt", bufs=2) as p3mt, \
                 tc.tile_pool(name="p4sb", bufs=3) as p4sb:
                def emit_k_group(sc, dm):
                    kp = paux.tile([128, 512], F32, tag="aux",
                                   name=f"kp{sc}_{dm}")
                    for lk in range(NLK):
                        nc.tensor.matmul(
                            kp[:],
                            wk_sb[:, lk, dm * 128:(dm + 1) * 128],
                            lat_sb[:, lk, sc * 512:(sc + 1) * 512],
                            start=(lk == 0), stop=(lk == NLK - 1))
                    nc.vector.tensor_scalar_add(
                        kt_sb[:, dm, sc * 512:(sc + 1) * 512], kp[:],
                        bk_sb[:, dm:dm + 1])

                def emit_v_group(sc, ti):
                    t = sc * 4 + ti
                    vp = paux.tile([128, 512], F32, tag="aux", name=f"vp{t}")
                    for lk in range(NLK):
                        nc.tensor.matmul(
                            vp[:],
                            lat_sb[:, lk, t * 128:(t + 1) * 128],
                            wv_sb[:, lk, :],
                            start=(lk == 0), stop=(lk == NLK - 1))
                    nc.scalar.copy(v_sb[:, t, :], vp[:])

                def emit_ph4_group(jj, gi):
                    # group gi in 0..15 of output chunk jj: 4 head matmuls
                    t = 4 * jj + gi // NJ
                    yc = gi % NJ
                    yp = paux.tile([128, 512], F32, tag="aux",
                                   name=f"yp{jj}_{gi}")
                    for h in range(HPC):
                        nc.tensor.matmul(
                            yp[:],
                            ots_sb[:, h, t * 128:(t + 1) * 128],
                            wo_sb[:, h, yc * 512:(yc + 1) * 512],
                            start=(h == 0), stop=(h == HPC - 1))
                    ys = p4sb.tile([128, 512], BF16, tag="ys")
                    if gi % 2 == 0:
                        nc.scalar.copy(ys[:], yp[:])
                    else:
                        nc.vector.tensor_copy(ys[:], yp[:])
                    nc.sync.dma_start(
                        out=y[t * 128:(t + 1) * 128,
                              yc * 512:(yc + 1) * 512],
                        in_=ys[:])

                def emit_ph3_group(j, h, n_i2, fillers=()):
                    # fillers: list of thunks emitting PE-heavy work, spread
                    # through the i2 loop to cover the ACT-bound exp cadence
                    fillers = list(fillers)
                    every = max(1, n_i2 // max(1, len(fillers))) if fillers else 0
                    ot = p3ot.tile([128, 512], F32, tag="ot")
                    acc = p3ac.tile([128, 512], FP16, tag="acc")
                    for i2 in range(n_i2):
                        if fillers and every and i2 % every == every - 1:
                            fillers.pop(0)()
                        r = i2 - 4 * j if causal else -1
                        off = 128 * r if r > 0 else 0
                        st = p3st.tile([128, 512], F32, tag="st")
                        nc.tensor.matmul(
                            st[:, off:512],
                            kt_sb[:, h, i2 * 128:(i2 + 1) * 128],
                            qt_sb[:, h, j * 512 + off:(j + 1) * 512],
                            start=True, stop=True)
                        if causal:
                            if r >= 0:
                                nc.vector.tensor_add(
                                    st[:, off:off + 128],
                                    st[:, off:off + 128], mb_sb[:])
                        else:
                            mt = p3mt.tile([128, 512], F32, tag="mt")
                            nc.sync.dma_start(
                                out=mt,
                                in_=maskb.rearrange(
                                    "(i p) q -> i p q", p=128)
                                [i2, :, j * 512:(j + 1) * 512])
                            nc.vector.tensor_add(st[:], st[:], mt[:])
                        et = p3et.tile([128, 512], FP16, tag="et")
                        nc.scalar.activation(et[:, off:512], st[:, off:512],
                                             EXP, scale=SCALE)
                        if i2 == 0:
                            nc.vector.tensor_copy(acc[:], et[:])
                        else:
                            nc.vector.tensor_add(
                                acc[:, off:512], acc[:, off:512],
                                et[:, off:512])
                        nc.tensor.matmul(
                            ot[:, off:512],
                            v_sb[:, i2, h * 128:(h + 1) * 128],
                            et[:, off:512],
                            start=(i2 == 0), stop=(i2 == n_i2 - 1))
                    cs = p3st.tile([128, 512], F32, tag="st", name="cs_st")
                    nc.tensor.matmul(cs[0:1, :], oneskh[:, 0:1],
                                     acc[:], start=True, stop=True)
                    csb = p3sb.tile([1, 512], FP16, tag="csb")
                    nc.vector.tensor_copy(csb[0:1, :], cs[0:1, :])
                    rb = p3st.tile([128, 512], F32, tag="st", name="rb_st")
                    nc.tensor.matmul(rb[:], ones1h[0:1, :],
                                     csb[0:1, :], start=True, stop=True)
                    rs = p3sb.tile([128, 512], F32, tag="rs")
                    nc.vector.reciprocal_approx_fast(out=rs[:], in_=rb[:])
                    nc.vector.tensor_mul(
                        ots_sb[:, h, j * 512:(j + 1) * 512], ot[:], rs[:])
                    for f in fillers:
                        f()

                if causal:
                    for rnd in range(NJ):
                        for ti in range(4):
                            emit_v_group(rnd, ti)
                        for h in range(HPC):
                            # K for head h of this round, plus the previous
                            # chunk's output-projection groups as PE filler.
                            # At rnd 0 every k-tile is diagonal, so K must be
                            # emitted before the ph3 group (PE queue order).
                            if rnd == 0:
                                emit_k_group(rnd, h)
                                fill = []
                            else:
                                fill = [lambda d=h, r=rnd: emit_k_group(r, d)]
                                fill += [
                                    lambda g=gi, r=rnd: emit_ph4_group(r - 1, g)
                                    for gi in range(4 * h, 4 * h + 4)]
                            emit_ph3_group(rnd, h, 4 * rnd + 4, fill)
                    for gi in range(16):
                        emit_ph4_group(NJ - 1, gi)
                else:
                    for sc in range(NJ):
                        for ti in range(4):
                            emit_v_group(sc, ti)
                        for dm in range(HPC):
                            emit_k_group(sc, dm)
                    for j in range(NJ):
                        for h in range(HPC):
                            emit_ph3_group(j, h, NQT)
                        for gi in range(16):
                            emit_ph4_group(j, gi)

    nc.compile()
    _BUILD_CACHE[causal] = nc
    return nc


def kernel(**inputs) -> np.ndarray:
    x = np.asarray(inputs["x"], dtype=np.float32)
    mask = np.asarray(inputs["mask"])
    Wq = np.asarray(inputs["Wq"], dtype=np.float32)
    bq = np.asarray(inputs["bq"], dtype=np.float32)
    Wkv = np.asarray(inputs["Wkv"], dtype=np.float32)
    bkv = np.asarray(inputs["bkv"], dtype=np.float32)
    Wk = np.asarray(inputs["Wk"], dtype=np.float32)
    bk = np.asarray(inputs["bk"], dtype=np.float32)
    Wv = np.asarray(inputs["Wv"], dtype=np.float32)
    bv = np.asarray(inputs["bv"], dtype=np.float32)
    Wo = np.asarray(inputs["Wo"], dtype=np.float32)
    bo = np.asarray(inputs["bo"], dtype=np.float32)

    tril = np.tril(np.ones((S, S), dtype=mask.dtype))
    causal = all(np.array_equal(mask[b], tril) for b in range(B))
    nc = build(causal)

    bf = lambda a: np.ascontiguousarray(a).astype(ml_dtypes.bfloat16)

    def wt(a, dt=None):
        # [K, M] -> [128, (K//128)*M]: contraction-tile-major, partition-first
        k, mm_ = a.shape
        out = np.ascontiguousarray(
            a.reshape(k // 128, 128, mm_).transpose(1, 0, 2).reshape(128, -1))
        return out.astype(dt) if dt is not None else bf(out)

    # triangular NEG bias for the 128-wide diagonal block: mask where f < p
    if causal:
        p = np.arange(128)[:, None]
        f = np.arange(128)[None, :]
        mb = np.where(f < p, NEG, 0.0).astype(np.float32)

    in_maps = []
    for c in range(N_CORES):
        b, g = divmod(c, HG)
        sl = slice(g * HSL, (g + 1) * HSL)
        m = {
            "xT": bf(x[b].T),
            "wq": bf(Wq[:, sl]),
            "bq": np.ascontiguousarray(bq[sl]).reshape(HPC, 128),
            "wkv": bf(Wkv),
            "bkv": bkv.reshape(NLK, 128),
            "wk": wt(Wk[:, sl]),
            "bk": np.ascontiguousarray(bk[sl]).reshape(HPC, 128),
            "wv": wt(Wv[:, sl]),
            "wo": wt(Wo[sl, :], np.float32),
        }
        if causal:
            m["maskb"] = mb
        else:
            m["maskb"] = np.ascontiguousarray(
                np.where(mask[b] == 0, NEG, 0.0).astype(np.float32))
        in_maps.append(m)

    res = run_bass_kernel_spmd(nc, in_maps, list(range(N_CORES)))
    bo_eff = (bo + bv @ Wo).astype(np.float32)
    out = np.empty((B, S, D_MODEL), dtype=np.float32)
    for b in range(B):
        acc = res.results[b * HG]["y"].astype(np.float32).copy()
        for g in range(1, HG):
            acc += res.results[b * HG + g]["y"]
        out[b] = acc + bo_eff
    return out
